# revision 19
# baseline (speedup 1.0000x reference)
"""Trainium2 Bass kernel for nn_AttentionBlockE3 (segment-softmax GNN attention).

Strategy: host sorts edges by destination node and partitions NODES across the
8 cores; nodes are packed into 128-node bins (10 bins/core) with greedy
balancing so every bin holds ~2500 edges -> T_fix=20 tiles of 128 edges.
All edge data streams as bf16 (half the HBM bytes of f32); cutoff/sqrt(60)
is folded into q on the host so no per-edge scale op is needed on device.

Per-core device program (SPMD, same NEFF on all 8 cores):
  chunk DMA  one [128, T*1448] bf16 transfer: per tile 960 qk cols
             (q' then k, head-contiguous 8x60) + 488 v cols
             ((d,h)-interleaved 61x8 with a ones-block at d=60)
  pass 1     per tile: prod = q'*k (DVE 2x), fold 60->30 (gpsimd),
             fold 30->15 (DVE), reduce 15 (DVE) -> logits w [128, 8] f32
  softmax    chunk-global max (DVE reduce + gpsimd partition allreduce),
             exp(w - C) on the scalar engine -> w_exp bf16
  pass 2     per tile: rhs = v_ext * w_exp broadcast (DVE 2x, yields both
             weighted v and the denominator columns), one-hot(dst) built on
             gpsimd, bf16 matmuls accumulate [128 nodes, 488] in PSUM
  epilog     out = psum[:, d,h] * recip(denom) -> bf16, DMA out
Host unpermutes columns, upcasts to f32, scatters rows to node ids.
"""
import numpy as np

E, D, N, H = 200000, 480, 10000, 8
P = 128
NCORES = 8
CHUNKS = 10                      # bins per core
NBINS = NCORES * CHUNKS
SCALE = 1.0 / np.sqrt(60.0)

# fused-col -> head-contiguous permutation (see reference irrep layout)
_BLOCKS = [(0, 16), (128, 24), (320, 20)]   # (fused offset, per-head width)


def _perms():
    perm_qk = np.empty(480, np.int64)       # hcontig[h*60+j] = fused[perm_qk]
    j0 = 0
    for off, hd in _BLOCKS:
        for h in range(H):
            for j in range(hd):
                perm_qk[h * 60 + j0 + j] = off + h * hd + j
        j0 += hd
    perm_v = np.empty(480, np.int64)        # vint[d*8+h] = fused[perm_v]
    for d in range(60):
        for h in range(H):
            perm_v[d * 8 + h] = perm_qk[h * 60 + d]
    return perm_qk, perm_v


PERM_QK, PERM_V = _perms()


def _plan(dst):
    """Balanced node->bin assignment. Returns per-(core,chunk) edge lists,
    node lists, and T_fix."""
    deg = np.bincount(dst, minlength=N)
    order = np.argsort(-deg, kind="stable")
    bin_edges = np.zeros(NBINS, np.int64)
    bin_nodes = np.zeros(NBINS, np.int64)
    node_bin = np.empty(N, np.int64)
    node_row = np.empty(N, np.int64)
    # greedy: highest-degree node to least-loaded bin with node capacity
    for n in order:
        open_mask = bin_nodes < P
        b = np.flatnonzero(open_mask)[np.argmin(bin_edges[open_mask])]
        node_bin[n] = b
        node_row[n] = bin_nodes[b]
        bin_nodes[b] += 1
        bin_edges[b] += deg[n]
    T_fix = int(np.max((bin_edges + P - 1) // P))
    T_fix = max(T_fix, 1)
    budget = T_fix * P
    # edge slots per bin
    eb = node_bin[dst]
    order_e = np.argsort(eb, kind="stable")
    gi = np.full((NBINS, budget), -1, np.int64)
    starts = np.searchsorted(eb[order_e], np.arange(NBINS))
    ends = np.searchsorted(eb[order_e], np.arange(NBINS), side="right")
    for b in range(NBINS):
        cnt = ends[b] - starts[b]
        gi[b, :cnt] = order_e[starts[b]:ends[b]]
    # node ids per bin row (for output scatter)
    nodes_of = np.full((NBINS, P), -1, np.int64)
    nodes_of[node_bin, node_row] = np.arange(N)
    return {"gi": gi.reshape(NCORES, CHUNKS, budget),
            "nodes_of": nodes_of.reshape(NCORES, CHUNKS, P),
            "node_row": node_row, "node_bin": node_bin, "T_fix": T_fix}


def _pack_core(core, plan, key, value, query, cutoff, dst):
    import ml_dtypes
    bf16 = ml_dtypes.bfloat16
    T_fix = plan["T_fix"]
    budget = T_fix * P
    g = plan["gi"][core].reshape(-1)          # [CHUNKS*budget]
    pad = g < 0
    gc = np.clip(g, 0, E - 1)
    q = (query[gc][:, PERM_QK] *
         (cutoff[gc] * SCALE)[:, None]).astype(np.float32)
    q[pad] = 0.0
    k = key[gc][:, PERM_QK]
    v = value[gc][:, PERM_V]
    n_slots = CHUNKS * budget
    qk = np.empty((n_slots, 960), dtype=bf16)
    qk[:, 0:480] = q.astype(bf16)
    qk[:, 480:960] = k.astype(bf16)
    # v block: [61, 8] (d,h)-interleaved with d-slot 60 = ones (denominator)
    ve = np.ones((n_slots, 61, 8), np.float32)
    ve[:, :60, :] = v.reshape(n_slots, 60, 8)
    vx = ve.reshape(n_slots, 488).astype(bf16)
    # row-in-bin of each edge's destination (or -5 for padding)
    dstrel = plan["node_row"][dst[gc]].astype(np.float32)
    dstrel[pad] = -5.0
    T_tot = CHUNKS * T_fix
    dstr = np.ascontiguousarray(
        dstrel.reshape(T_tot, P).T).astype(np.float32)    # [128, T_tot]
    # chunk-major DMA layouts [CHUNKS*128, T_fix*width]
    qk = np.ascontiguousarray(
        qk.reshape(CHUNKS, T_fix, P, 960).transpose(0, 2, 1, 3)
          .reshape(CHUNKS * P, T_fix * 960))
    vx = np.ascontiguousarray(
        vx.reshape(CHUNKS, T_fix, P, 488).transpose(0, 2, 1, 3)
          .reshape(CHUNKS * P, T_fix * 488))
    return {"qk": qk, "v": vx, "dstr": dstr}


def _build_program(T_fix, CHUNKS_, reps=1, **_ignored):
    import contextlib

    import concourse.bacc as bacc
    import concourse.mybir as mybir
    import concourse.tile as tile
    from concourse import bass_isa

    f32 = mybir.dt.float32
    bf16 = mybir.dt.bfloat16
    T = T_fix
    T_tot = CHUNKS_ * T
    WQ, WV = T * 960, T * 488

    nc = bacc.Bacc("TRN2", target_bir_lowering=False, debug=False,
                   num_devices=NCORES)
    qk_d = nc.dram_tensor("qk", [CHUNKS_ * P, WQ], bf16,
                          kind="ExternalInput").ap()
    v_d = nc.dram_tensor("v", [CHUNKS_ * P, WV], bf16,
                         kind="ExternalInput").ap()
    dstr_d = nc.dram_tensor("dstr", [P, T_tot], f32,
                            kind="ExternalInput").ap()
    out_d = nc.dram_tensor("out", [CHUNKS_ * P, 480], bf16,
                           kind="ExternalOutput").ap()

    with tile.TileContext(nc) as tc:
        with (
            tc.tile_pool(name="const", bufs=1) as const_pool,
            tc.tile_pool(name="qkp", bufs=2) as qk_pool,
            tc.tile_pool(name="vp", bufs=3) as v_pool,
            tc.tile_pool(name="prod", bufs=1) as prod_pool,
            tc.tile_pool(name="f1", bufs=1) as f1_pool,
            tc.tile_pool(name="f2", bufs=1) as f2_pool,
            tc.tile_pool(name="w", bufs=2) as w_pool,
            tc.tile_pool(name="wexp", bufs=2) as wexp_pool,
            tc.tile_pool(name="oh", bufs=3) as oh_pool,
            tc.tile_pool(name="rhs", bufs=2) as rhs_pool,
            tc.tile_pool(name="stat", bufs=4) as stat_pool,
            tc.tile_pool(name="outp", bufs=2) as out_pool,
            tc.tile_pool(name="psum", bufs=2, space="PSUM") as psum_pool,
        ):
            iota_i = const_pool.tile([P, P], mybir.dt.int32)
            nc.gpsimd.iota(iota_i[:], pattern=[[1, P]], base=0,
                           channel_multiplier=0)
            iota_b = const_pool.tile([P, P], bf16)
            nc.vector.tensor_copy(iota_b[:], iota_i[:])
            dstr_sb = const_pool.tile([P, T_tot], f32)
            nc.sync.dma_start(out=dstr_sb[:], in_=dstr_d[:, :])

            def dma_in(c):
                qk = qk_pool.tile([P, WQ], bf16)
                nc.sync.dma_start(out=qk[:], in_=qk_d[c * P:(c + 1) * P, :])
                v = v_pool.tile([P, WV], bf16)
                nc.scalar.dma_start(out=v[:], in_=v_d[c * P:(c + 1) * P, :])
                return qk, v

            def pass1(qk):
                # whole-chunk q*k + per-head tree reduction, 4 wide DVE ops
                qk3 = qk[:].rearrange("p (t j) -> p t j", t=T)
                prod = prod_pool.tile([P, T * 480], bf16)
                pv = prod[:].rearrange("p (t h d) -> p t h d", t=T, h=H)
                nc.vector.tensor_mul(
                    pv.rearrange("p t h d -> p t (h d)"),
                    qk3[:, :, 0:480], qk3[:, :, 480:960])
                f1 = f1_pool.tile([P, T * 240], bf16)
                f1v = f1[:].rearrange("p (t h d) -> p t h d", t=T, h=H)
                nc.vector.tensor_add(f1v, pv[:, :, :, 0:30],
                                     pv[:, :, :, 30:60])
                f2 = f2_pool.tile([P, T * 120], bf16)
                f2v = f2[:].rearrange("p (t h d) -> p t h d", t=T, h=H)
                nc.vector.tensor_add(f2v, f1v[:, :, :, 0:15],
                                     f1v[:, :, :, 15:30])
                w_f = w_pool.tile([P, T * 8], f32)
                nc.vector.reduce_sum(
                    out=w_f[:].rearrange("p (t h) -> p t h", t=T),
                    in_=f2v, axis=mybir.AxisListType.X)
                return w_f

            def stats(w_f):
                # chunk-global max -> -C -> exp; only reduce_max is on DVE,
                # the rest rides Pool/Act so DVE can start the next pass 1
                wmax = stat_pool.tile([P, 1], f32)
                nc.vector.reduce_max(out=wmax[:], in_=w_f[:],
                                     axis=mybir.AxisListType.X)
                cmax = stat_pool.tile([P, 1], f32)
                nc.gpsimd.partition_all_reduce(cmax[:], wmax[:], channels=P,
                                               reduce_op=bass_isa.ReduceOp.max)
                negC = stat_pool.tile([P, 1], f32)
                nc.scalar.mul(negC[:], cmax[:], -1.0)
                wexp = wexp_pool.tile([P, T * 8], bf16)
                nc.scalar.activation(wexp[:], w_f[:],
                                     mybir.ActivationFunctionType.Exp,
                                     bias=negC[:], scale=1.0)
                return wexp

            def one_hots(c):
                ohs = []
                for t in range(T):
                    oh = oh_pool.tile([P, P], bf16)
                    nc.vector.tensor_scalar(
                        oh[:], iota_b[:],
                        dstr_sb[:, c * T + t:c * T + t + 1], None,
                        op0=mybir.AluOpType.is_equal)
                    ohs.append(oh)
                return ohs

            def pass2_mm(v, wexp, ohs):
                psum_t = psum_pool.tile([P, 488], f32)
                v3 = v[:].rearrange("p (u j) -> p u j", u=T)
                GMAX = 5
                for t in range(T):
                    if t % GMAX == 0:
                        g_n = min(GMAX, T - t)
                        rhs = rhs_pool.tile([P, g_n * 488], bf16)
                        nc.vector.tensor_mul(
                            rhs[:].rearrange("p (g d h) -> p g d h",
                                             g=g_n, h=H),
                            v3[:, t:t + g_n, :]
                                .rearrange("p g (d h) -> p g d h", h=H),
                            wexp[:, t * 8:(t + g_n) * 8]
                                .rearrange("p (g h) -> p g h", g=g_n)
                                .unsqueeze(2).to_broadcast([P, g_n, 61, H]))
                    gg = t % GMAX
                    nc.tensor.matmul(out=psum_t[:], lhsT=ohs[t][:],
                                     rhs=rhs[:, gg * 488:(gg + 1) * 488],
                                     start=(t == 0), stop=(t == T - 1))
                return psum_t

            def epilogue(c, psum_t):
                # normalize: recip on DVE, per-head scale on Act engine
                srec = stat_pool.tile([P, 8], f32)
                nc.vector.tensor_scalar_add(srec[:], psum_t[:, 480:488],
                                            1e-30)
                nc.vector.reciprocal(srec[:], srec[:])
                outt = out_pool.tile([P, 480], bf16)
                for h in range(H):
                    nc.scalar.mul(
                        outt[:].rearrange("p (d h) -> p d h", h=H)[:, :, h],
                        psum_t[:, 0:480]
                            .rearrange("p (d h) -> p d h", h=H)[:, :, h],
                        srec[:, h:h + 1])
                nc.sync.dma_start(out=out_d[c * P:(c + 1) * P, :],
                                  in_=outt[:])

            def body():
                # software-pipelined: stats/oh/matmuls of chunk c interleave
                # with DMA + pass 1 of chunk c+1; the PSUM drain (epilogue)
                # of chunk c is deferred into iteration c+1 so DVE never
                # waits on the tail matmuls
                qk, v = dma_in(0)
                w_f = pass1(qk)
                live = (v, w_f)
                pend = None
                for c in range(CHUNKS_):
                    v, w_f = live
                    if c + 1 < CHUNKS_:
                        qk_n, v_n = dma_in(c + 1)
                    wexp = stats(w_f)
                    ohs = one_hots(c)
                    if pend is not None:
                        epilogue(c - 1, pend)
                    if c + 1 < CHUNKS_:
                        live = (v_n, pass1(qk_n))
                    pend = pass2_mm(v, wexp, ohs)
                epilogue(CHUNKS_ - 1, pend)

            loop = tc.For_i(0, reps, 1) if reps > 1 else contextlib.nullcontext()
            with loop:
                body()

    nc.compile()
    return nc


def _postprocess(outs, plan):
    """outs: list of per-core 'out' arrays [CHUNKS*128, 480] (bf16/f32).
    Returns full [N, 480] f32 in the reference fused layout."""
    full = np.zeros((N, 480), np.float32)
    for core in range(NCORES):
        o = np.asarray(outs[core], dtype=np.float32).reshape(CHUNKS, P, 480)
        for c in range(CHUNKS):
            ids = plan["nodes_of"][core, c]
            m = ids >= 0
            full[ids[m][:, None], PERM_V[None, :]] = o[c, m, :]
    return full


def kernel(key, value, query, edge_weight_cutoff, edge_index, num_nodes):
    key = np.ascontiguousarray(np.asarray(key, dtype=np.float32))
    value = np.ascontiguousarray(np.asarray(value, dtype=np.float32))
    query = np.ascontiguousarray(np.asarray(query, dtype=np.float32))
    cutoff = np.asarray(edge_weight_cutoff, dtype=np.float32)
    dst = np.asarray(edge_index)[1].astype(np.int64)

    plan = _plan(dst)
    in_maps = [_pack_core(core, plan, key, value, query, cutoff, dst)
               for core in range(NCORES)]
    nc = _build_program(plan["T_fix"], CHUNKS)

    from concourse.bass_utils import run_bass_kernel_spmd
    res = run_bass_kernel_spmd(nc, in_maps, core_ids=list(range(NCORES)))
    return _postprocess([r["out"] for r in res.results], plan)


if __name__ == "__main__":
    rng = np.random.default_rng(0)
    inputs = {
        "key": rng.standard_normal((E, D)).astype(np.float32),
        "value": rng.standard_normal((E, D)).astype(np.float32),
        "query": rng.standard_normal((E, D)).astype(np.float32),
        "edge_weight_cutoff": rng.random(E).astype(np.float32),
        "edge_index": rng.integers(0, N, (2, E)),
        "num_nodes": N,
    }
    out = kernel(**inputs)
    print("out", out.shape, out.dtype, float(np.abs(out).max()))


# revision 21
# speedup vs baseline: 1.1167x; 1.1167x over previous
"""Trainium2 Bass kernel for nn_AttentionBlockE3 (segment-softmax GNN attention).

Strategy: host partitions NODES across the 8 cores via balanced 128-node bins
(10 bins/core; greedy by degree -> every bin holds ~2500 edges = 20 tiles of
128 edges), so each core owns all edges of its nodes and no collectives are
needed. All edge data streams as bf16 (half the HBM bytes of f32; the kernel
is HBM-bound at ~75 MB/core). cutoff/sqrt(60) is folded into q on the host.

Per-core device program (SPMD, same NEFF on all 8 cores), software-pipelined
per 128-node chunk (stats/pass2 of chunk c run while chunk c+1 streams in and
its pass 1 occupies the vector engine):
  DMA      qk [128, T*960] bf16 (q' then k, head-contiguous 8x60) on the SP
           ring; v [128, T*488] bf16 ((d,h)-interleaved 61x8, ones at d=60
           for the denominator) on the Act ring
  pass 1   whole-chunk q'*k + per-head tree reduction in 4 wide DVE ops
           (mult 2x-bf16, fold 60->30, fold 30->15, reduce 15) -> w [128,T*8]
  softmax  chunk-global max: DVE reduce_max + gpsimd partition allreduce;
           -C and exp(w - C) on the scalar engine -> w_exp bf16
  pass 2   rhs = v_ext * w_exp broadcast (5-tile-wide DVE ops, yields both
           weighted v and denominator columns); one-hot(dst) per tile via
           tensor_scalar(is_equal) at 4x; bf16 matmuls accumulate
           [128 nodes, 488] in PSUM
  epilog   recip(denom) on DVE; per-head scale on the scalar engine -> bf16
Host unpermutes columns, upcasts to f32, scatters rows to node ids.
"""
import numpy as np

E, D, N, H = 200000, 480, 10000, 8
P = 128
NCORES = 8
CHUNKS = 10                      # bins per core
NBINS = NCORES * CHUNKS
SCALE = 1.0 / np.sqrt(60.0)

# fused-col -> head-contiguous permutation (see reference irrep layout)
_BLOCKS = [(0, 16), (128, 24), (320, 20)]   # (fused offset, per-head width)


def _perms():
    perm_qk = np.empty(480, np.int64)       # hcontig[h*60+j] = fused[perm_qk]
    j0 = 0
    for off, hd in _BLOCKS:
        for h in range(H):
            for j in range(hd):
                perm_qk[h * 60 + j0 + j] = off + h * hd + j
        j0 += hd
    perm_v = np.empty(480, np.int64)        # vint[d*8+h] = fused[perm_v]
    for d in range(60):
        for h in range(H):
            perm_v[d * 8 + h] = perm_qk[h * 60 + d]
    return perm_qk, perm_v


PERM_QK, PERM_V = _perms()


def _plan(dst):
    """Balanced node->bin assignment. Returns per-(core,chunk) edge lists,
    node lists, and T_fix."""
    deg = np.bincount(dst, minlength=N)
    order = np.argsort(-deg, kind="stable")
    bin_edges = np.zeros(NBINS, np.int64)
    bin_nodes = np.zeros(NBINS, np.int64)
    node_bin = np.empty(N, np.int64)
    node_row = np.empty(N, np.int64)
    # greedy: highest-degree node to least-loaded bin with node capacity
    for n in order:
        open_mask = bin_nodes < P
        b = np.flatnonzero(open_mask)[np.argmin(bin_edges[open_mask])]
        node_bin[n] = b
        node_row[n] = bin_nodes[b]
        bin_nodes[b] += 1
        bin_edges[b] += deg[n]
    T_fix = int(np.max((bin_edges + P - 1) // P))
    T_fix = max(T_fix, 1)
    budget = T_fix * P
    # edge slots per bin
    eb = node_bin[dst]
    order_e = np.argsort(eb, kind="stable")
    gi = np.full((NBINS, budget), -1, np.int64)
    starts = np.searchsorted(eb[order_e], np.arange(NBINS))
    ends = np.searchsorted(eb[order_e], np.arange(NBINS), side="right")
    for b in range(NBINS):
        cnt = ends[b] - starts[b]
        gi[b, :cnt] = order_e[starts[b]:ends[b]]
    # node ids per bin row (for output scatter)
    nodes_of = np.full((NBINS, P), -1, np.int64)
    nodes_of[node_bin, node_row] = np.arange(N)
    return {"gi": gi.reshape(NCORES, CHUNKS, budget),
            "nodes_of": nodes_of.reshape(NCORES, CHUNKS, P),
            "node_row": node_row, "node_bin": node_bin, "T_fix": T_fix}


def _pack_core(core, plan, key, value, query, cutoff, dst):
    import ml_dtypes
    bf16 = ml_dtypes.bfloat16
    T_fix = plan["T_fix"]
    budget = T_fix * P
    g = plan["gi"][core].reshape(-1)          # [CHUNKS*budget]
    pad = g < 0
    gc = np.clip(g, 0, E - 1)
    q = (query[gc][:, PERM_QK] *
         (cutoff[gc] * SCALE)[:, None]).astype(np.float32)
    q[pad] = 0.0
    k = key[gc][:, PERM_QK]
    v = value[gc][:, PERM_V]
    n_slots = CHUNKS * budget
    qk = np.empty((n_slots, 960), dtype=bf16)
    qk[:, 0:480] = q.astype(bf16)
    qk[:, 480:960] = k.astype(bf16)
    # v block: [61, 8] (d,h)-interleaved with d-slot 60 = ones (denominator)
    ve = np.ones((n_slots, 61, 8), np.float32)
    ve[:, :60, :] = v.reshape(n_slots, 60, 8)
    vx = ve.reshape(n_slots, 488).astype(bf16)
    # row-in-bin of each edge's destination (or -5 for padding)
    dstrel = plan["node_row"][dst[gc]].astype(np.float32)
    dstrel[pad] = -5.0
    T_tot = CHUNKS * T_fix
    dstr = np.ascontiguousarray(
        dstrel.reshape(T_tot, P).T).astype(np.float32)    # [128, T_tot]
    # chunk-major DMA layouts [CHUNKS*128, T_fix*width]
    qk = np.ascontiguousarray(
        qk.reshape(CHUNKS, T_fix, P, 960).transpose(0, 2, 1, 3)
          .reshape(CHUNKS * P, T_fix * 960))
    vx = np.ascontiguousarray(
        vx.reshape(CHUNKS, T_fix, P, 488).transpose(0, 2, 1, 3)
          .reshape(CHUNKS * P, T_fix * 488))
    return {"qk": qk, "v": vx, "dstr": dstr}


def _build_program(T_fix, CHUNKS_, reps=1, **_ignored):
    import contextlib

    import concourse.bacc as bacc
    import concourse.mybir as mybir
    import concourse.tile as tile
    from concourse import bass_isa

    f32 = mybir.dt.float32
    bf16 = mybir.dt.bfloat16
    T = T_fix
    T_tot = CHUNKS_ * T
    WQ, WV = T * 960, T * 488

    nc = bacc.Bacc("TRN2", target_bir_lowering=False, debug=False,
                   num_devices=NCORES)
    qk_d = nc.dram_tensor("qk", [CHUNKS_ * P, WQ], bf16,
                          kind="ExternalInput").ap()
    v_d = nc.dram_tensor("v", [CHUNKS_ * P, WV], bf16,
                         kind="ExternalInput").ap()
    dstr_d = nc.dram_tensor("dstr", [P, T_tot], f32,
                            kind="ExternalInput").ap()
    out_d = nc.dram_tensor("out", [CHUNKS_ * P, 480], bf16,
                           kind="ExternalOutput").ap()

    with tile.TileContext(nc) as tc:
        with (
            tc.tile_pool(name="const", bufs=1) as const_pool,
            tc.tile_pool(name="qkp", bufs=2) as qk_pool,
            tc.tile_pool(name="vp", bufs=3) as v_pool,
            tc.tile_pool(name="prod", bufs=1) as prod_pool,
            tc.tile_pool(name="f1", bufs=1) as f1_pool,
            tc.tile_pool(name="f2", bufs=1) as f2_pool,
            tc.tile_pool(name="w", bufs=2) as w_pool,
            tc.tile_pool(name="wexp", bufs=2) as wexp_pool,
            tc.tile_pool(name="oh", bufs=3) as oh_pool,
            tc.tile_pool(name="rhs", bufs=2) as rhs_pool,
            tc.tile_pool(name="stat", bufs=4) as stat_pool,
            tc.tile_pool(name="outp", bufs=2) as out_pool,
            tc.tile_pool(name="psum", bufs=2, space="PSUM") as psum_pool,
        ):
            iota_i = const_pool.tile([P, P], mybir.dt.int32)
            nc.gpsimd.iota(iota_i[:], pattern=[[1, P]], base=0,
                           channel_multiplier=0)
            iota_b = const_pool.tile([P, P], bf16)
            nc.vector.tensor_copy(iota_b[:], iota_i[:])
            dstr_sb = const_pool.tile([P, T_tot], f32)
            nc.sync.dma_start(out=dstr_sb[:], in_=dstr_d[:, :])

            def dma_in(c):
                qk = qk_pool.tile([P, WQ], bf16)
                nc.sync.dma_start(out=qk[:], in_=qk_d[c * P:(c + 1) * P, :])
                v = v_pool.tile([P, WV], bf16)
                nc.scalar.dma_start(out=v[:], in_=v_d[c * P:(c + 1) * P, :])
                return qk, v

            def pass1(qk):
                # whole-chunk q*k + per-head tree reduction, 4 wide DVE ops
                qk3 = qk[:].rearrange("p (t j) -> p t j", t=T)
                prod = prod_pool.tile([P, T * 480], bf16)
                pv = prod[:].rearrange("p (t h d) -> p t h d", t=T, h=H)
                nc.vector.tensor_mul(
                    pv.rearrange("p t h d -> p t (h d)"),
                    qk3[:, :, 0:480], qk3[:, :, 480:960])
                f1 = f1_pool.tile([P, T * 240], bf16)
                f1v = f1[:].rearrange("p (t h d) -> p t h d", t=T, h=H)
                nc.vector.tensor_add(f1v, pv[:, :, :, 0:30],
                                     pv[:, :, :, 30:60])
                f2 = f2_pool.tile([P, T * 120], bf16)
                f2v = f2[:].rearrange("p (t h d) -> p t h d", t=T, h=H)
                nc.vector.tensor_add(f2v, f1v[:, :, :, 0:15],
                                     f1v[:, :, :, 15:30])
                w_f = w_pool.tile([P, T * 8], f32)
                nc.vector.reduce_sum(
                    out=w_f[:].rearrange("p (t h) -> p t h", t=T),
                    in_=f2v, axis=mybir.AxisListType.X)
                return w_f

            def stats(w_f):
                # chunk-global max -> -C -> exp; only reduce_max is on DVE,
                # the rest rides Pool/Act so DVE can start the next pass 1
                wmax = stat_pool.tile([P, 1], f32)
                nc.vector.reduce_max(out=wmax[:], in_=w_f[:],
                                     axis=mybir.AxisListType.X)
                cmax = stat_pool.tile([P, 1], f32)
                nc.gpsimd.partition_all_reduce(cmax[:], wmax[:], channels=P,
                                               reduce_op=bass_isa.ReduceOp.max)
                negC = stat_pool.tile([P, 1], f32)
                nc.scalar.mul(negC[:], cmax[:], -1.0)
                wexp = wexp_pool.tile([P, T * 8], bf16)
                nc.scalar.activation(wexp[:], w_f[:],
                                     mybir.ActivationFunctionType.Exp,
                                     bias=negC[:], scale=1.0)
                return wexp

            def one_hots(c):
                ohs = []
                for t in range(T):
                    oh = oh_pool.tile([P, P], bf16)
                    nc.vector.tensor_scalar(
                        oh[:], iota_b[:],
                        dstr_sb[:, c * T + t:c * T + t + 1], None,
                        op0=mybir.AluOpType.is_equal)
                    ohs.append(oh)
                return ohs

            def pass2(c, v, wexp, ohs):
                psum_t = psum_pool.tile([P, 488], f32)
                v3 = v[:].rearrange("p (u j) -> p u j", u=T)
                GMAX = 5
                for t in range(T):
                    if t % GMAX == 0:
                        g_n = min(GMAX, T - t)
                        rhs = rhs_pool.tile([P, g_n * 488], bf16)
                        nc.vector.tensor_mul(
                            rhs[:].rearrange("p (g d h) -> p g d h",
                                             g=g_n, h=H),
                            v3[:, t:t + g_n, :]
                                .rearrange("p g (d h) -> p g d h", h=H),
                            wexp[:, t * 8:(t + g_n) * 8]
                                .rearrange("p (g h) -> p g h", g=g_n)
                                .unsqueeze(2).to_broadcast([P, g_n, 61, H]))
                    gg = t % GMAX
                    nc.tensor.matmul(out=psum_t[:], lhsT=ohs[t][:],
                                     rhs=rhs[:, gg * 488:(gg + 1) * 488],
                                     start=(t == 0), stop=(t == T - 1))
                # epilogue: normalize on the scalar engine per head
                srec = stat_pool.tile([P, 8], f32)
                nc.vector.tensor_scalar_add(srec[:], psum_t[:, 480:488],
                                            1e-30)
                nc.vector.reciprocal(srec[:], srec[:])
                outt = out_pool.tile([P, 480], bf16)
                for h in range(H):
                    nc.scalar.mul(
                        outt[:].rearrange("p (d h) -> p d h", h=H)[:, :, h],
                        psum_t[:, 0:480]
                            .rearrange("p (d h) -> p d h", h=H)[:, :, h],
                        srec[:, h:h + 1])
                nc.sync.dma_start(out=out_d[c * P:(c + 1) * P, :],
                                  in_=outt[:])

            def body():
                # software-pipelined: stats/oh/p2 of chunk c interleave with
                # DMA and pass 1 of chunk c+1 so Pool/Act latency hides
                # under DVE work
                qk, v = dma_in(0)
                w_f = pass1(qk)
                live = (v, w_f)
                for c in range(CHUNKS_):
                    v, w_f = live
                    if c + 1 < CHUNKS_:
                        qk_n, v_n = dma_in(c + 1)
                    wexp = stats(w_f)
                    ohs = one_hots(c)
                    if c + 1 < CHUNKS_:
                        live = (v_n, pass1(qk_n))
                    pass2(c, v, wexp, ohs)

            loop = tc.For_i(0, reps, 1) if reps > 1 else contextlib.nullcontext()
            with loop:
                body()

    nc.compile()
    return nc


def _postprocess(outs, plan):
    """outs: list of per-core 'out' arrays [CHUNKS*128, 480] (bf16/f32).
    Returns full [N, 480] f32 in the reference fused layout."""
    full = np.zeros((N, 480), np.float32)
    for core in range(NCORES):
        o = np.asarray(outs[core], dtype=np.float32).reshape(CHUNKS, P, 480)
        for c in range(CHUNKS):
            ids = plan["nodes_of"][core, c]
            m = ids >= 0
            full[ids[m][:, None], PERM_V[None, :]] = o[c, m, :]
    return full


def kernel(key, value, query, edge_weight_cutoff, edge_index, num_nodes):
    key = np.ascontiguousarray(np.asarray(key, dtype=np.float32))
    value = np.ascontiguousarray(np.asarray(value, dtype=np.float32))
    query = np.ascontiguousarray(np.asarray(query, dtype=np.float32))
    cutoff = np.asarray(edge_weight_cutoff, dtype=np.float32)
    dst = np.asarray(edge_index)[1].astype(np.int64)

    plan = _plan(dst)
    in_maps = [_pack_core(core, plan, key, value, query, cutoff, dst)
               for core in range(NCORES)]
    nc = _build_program(plan["T_fix"], CHUNKS)

    from concourse.bass_utils import run_bass_kernel_spmd
    res = run_bass_kernel_spmd(nc, in_maps, core_ids=list(range(NCORES)))
    return _postprocess([r["out"] for r in res.results], plan)


if __name__ == "__main__":
    rng = np.random.default_rng(0)
    inputs = {
        "key": rng.standard_normal((E, D)).astype(np.float32),
        "value": rng.standard_normal((E, D)).astype(np.float32),
        "query": rng.standard_normal((E, D)).astype(np.float32),
        "edge_weight_cutoff": rng.random(E).astype(np.float32),
        "edge_index": rng.integers(0, N, (2, E)),
        "num_nodes": N,
    }
    out = kernel(**inputs)
    print("out", out.shape, out.dtype, float(np.abs(out).max()))
